# revision 35
# baseline (speedup 1.0000x reference)
"""CFConv (GNN message passing) on 8 Trainium2 cores — streaming design.

    y = segment_sum(x[idx_j] * Wij, idx_i)   with idx_i sorted

The host pre-gathers x[idx_j] into edge-slot order (pure data movement),
and the device streams everything at line rate in bf16:

  - Edges sharded contiguously across 8 cores (idx_i sorted).
  - Per core, edges packed in order into 128-slot "halves" (atom span < 16;
    ~200 edges fall in any 16-atom window so halves stay ~100% full).
  - Per ~96-column chunk: x_ij streamed on the sync queue, Wij on the
    scalar queue.  Chunks are large because a [128, N] DMA costs 128
    descriptors at ~38ns HWDGE emission regardless of N, capping a queue
    at chunk_bytes/4.9us.  rr/iota and the out-writes ride the SWDGE
    (gpsimd) queue so the HWDGE queues stay input-only (any non-DMA wait
    on the issuing engine would FIFO-couple the stream to compute).
  - VectorE: one-hot (iota == rr) at 1x, then prod = x_ij * Wij (bf16 2x).
  - TensorE: per half, one K=128 M=16 N=64 matmul of one-hot^T @ prod;
    4 consecutive halves land in 4 distinct 32-aligned PSUM partition
    groups (tile_position col-tiling) and run concurrently on separate
    PE strips.
  - ScalarE copies PSUM->SBUF bf16, deferred 2 chunks in emission order
    so its PE-wait never blocks the wij DMA issues behind it; two chunks
    share a stage tile; per-group out-DMAs write rows 16g..16g+15.
  - Host adds the overlapping [16,F] partial frames per atom range (f32).
"""

import sys

import numpy as np

if "/opt/trn_rl_repo" not in sys.path:
    sys.path.insert(0, "/opt/trn_rl_repo")

CFG = dict(
    N_ATOMS=100000,
    F=64,
    E=1250000,
    NCORES=8,
    HALF=128,          # edge slots per half (1 col of 128)
    OHW=16,            # one-hot width (max atom span per half)
    NH=1224,           # halves per core (156,672 slots >= 156,250 edges)
    # chunk sizes in halves (= columns); taper both ends.  Big chunks matter:
    # HWDGE descriptor emission is ~38ns x 128 descs per DMA irrespective of
    # size, so per-queue bandwidth ~= chunk_bytes / 4.9us
    CHUNK_HALVES=[24] + [80] * 14 + [48, 32],
)

_CACHE = {}
last_results = None


def _derived(cfg):
    d = dict(cfg)
    d["CPH"] = cfg["HALF"] // 128      # cols per half
    d["CAP"] = cfg["NH"] * cfg["HALF"]
    d["NCOLS"] = d["CAP"] // 128
    assert sum(cfg["CHUNK_HALVES"]) == cfg["NH"]
    # halves pack 4-up onto PSUM partition groups (col-tiling)
    assert cfg["NH"] % 4 == 0
    assert all(nh % 4 == 0 for nh in cfg["CHUNK_HALVES"])
    return d


def _build_program(cfg):
    import concourse.bacc as bacc
    import concourse.tile as tile
    import concourse.mybir as mybir

    d = _derived(cfg)
    F = cfg["F"]
    NCOLS, CPH = d["NCOLS"], d["CPH"]
    NH, OHW = cfg["NH"], cfg["OHW"]
    MAXH = max(cfg["CHUNK_HALVES"])        # halves per chunk tile
    MAXC = MAXH * CPH                      # cols per chunk tile

    nc = bacc.Bacc("TRN2", target_bir_lowering=False)
    f32 = mybir.dt.float32
    bf16 = mybir.dt.bfloat16
    xij_d = nc.dram_tensor("xij", [128, NCOLS * F], bf16, kind="ExternalInput")
    wij_d = nc.dram_tensor("wij", [128, NCOLS * F], bf16, kind="ExternalInput")
    rr_d = nc.dram_tensor("rr", [128, NCOLS], bf16, kind="ExternalInput")
    iota_d = nc.dram_tensor("iota", [128, OHW], bf16, kind="ExternalInput")
    # half h's [OHW, F] frame: PSUM partitions [32*(h%4) : 32*(h%4)+OHW],
    # staged out to rows [16*(h%4) : 16*(h%4)+16] of out_d, cols
    # [(h//4)*F : (h//4+1)*F]  (4 halves share a 128-partition group)
    out_d = nc.dram_tensor("out", [4 * OHW, (NH // 4) * F], bf16, kind="ExternalOutput")

    with tile.TileContext(nc) as tc:
        with (
            tc.tile_pool(name="const", bufs=1) as cpool,
            tc.tile_pool(name="data", bufs=8) as dpool,
            tc.tile_pool(name="oh", bufs=4) as spool,
            tc.tile_pool(name="stage", bufs=3) as opool,
            tc.tile_pool(name="psum", bufs=2, space="PSUM") as ppool,
        ):
            # rr/iota ride the SWDGE (gpsimd) queue so the two HWDGE queues
            # start streaming edge data immediately after the preamble.
            iota_t = cpool.tile([128, OHW], bf16)
            nc.gpsimd.dma_start(out=iota_t[:], in_=iota_d[:])
            rr_t = cpool.tile([128, NCOLS], bf16)
            nc.gpsimd.dma_start(out=rr_t[:], in_=rr_d[:])

            iota_b = iota_t[:].rearrange("p (o f) -> p o f", o=1)

            chunks = cfg["CHUNK_HALVES"]
            nchunks = len(chunks)
            DEFER = 2   # emit chunk k's PSUM->SBUF copy during iteration k+2
            pend = {}   # ci -> (pt tile, nh, col0)
            pair = {"stage": None, "off": 0, "h0": 0}

            def emit_copy_out(cj):
                """Emit the deferred ACT copy (+ paired out-DMA flush) for
                chunk cj.  Deferring keeps the ACT engine's sequencer from
                blocking on PE while it still has input DMAs to issue."""
                pt_j, nh_j, c0_j = pend.pop(cj)
                if pair["stage"] is None:
                    pair["stage"] = opool.tile(
                        [128, (2 * MAXH // 4) * F], bf16, tag="st", name="stage"
                    )
                    pair["off"] = 0
                    pair["h0"] = c0_j
                nc.scalar.copy(
                    out=pair["stage"][
                        :, pair["off"] : pair["off"] + (nh_j // 4) * F
                    ],
                    in_=pt_j[:, : (nh_j // 4) * F],
                )
                pair["off"] += (nh_j // 4) * F
                last = cj == nchunks - 1
                if cj % 2 == 1 or last:
                    cr = slice((pair["h0"] // 4) * F, ((c0_j + nh_j) // 4) * F)
                    # per partition-group out-writes (rows 16..31 of each
                    # group are garbage).  They ride the SWDGE queue: HWDGE
                    # queues must stay input-only, else FIFO order couples
                    # the input stream to compute.  The final flush spreads
                    # over the (by then idle) HWDGE queues to cut the tail.
                    oqs = (
                        [nc.sync, nc.scalar, nc.gpsimd, nc.gpsimd]
                        if last
                        else [nc.gpsimd] * 4
                    )
                    for g in range(4):
                        oqs[g].dma_start(
                            out=out_d[16 * g : 16 * g + 16, cr],
                            in_=pair["stage"][32 * g : 32 * g + 16, : pair["off"]],
                        )
                    pair["stage"] = None

            col0 = 0   # global column base of chunk
            for ci, nh in enumerate(chunks):
                ncols = CPH * nh
                xt = dpool.tile([128, MAXC * F], bf16, tag="x")
                nc.sync.dma_start(
                    out=xt[:, : ncols * F],
                    in_=xij_d[:, col0 * F : (col0 + ncols) * F],
                )
                wt = dpool.tile([128, MAXC * F], bf16, tag="w")
                nc.scalar.dma_start(
                    out=wt[:, : ncols * F],
                    in_=wij_d[:, col0 * F : (col0 + ncols) * F],
                )
                # one-hot first: it depends only on rr, so it fills DVE time
                # while the chunk's data is still in flight
                s_t = spool.tile([128, MAXC * OHW], bf16, tag="oh")
                nc.vector.tensor_tensor(
                    out=s_t[:, : ncols * OHW],
                    in0=iota_b.to_broadcast([128, ncols, OHW]),
                    in1=rr_t[:, col0 : col0 + ncols].to_broadcast([128, ncols, OHW]),
                    op=mybir.AluOpType.is_equal,
                )
                nc.vector.tensor_tensor(
                    out=xt[:, : ncols * F],
                    in0=xt[:, : ncols * F],
                    in1=wt[:, : ncols * F],
                    op=mybir.AluOpType.mult,
                )
                # 4 halves fan out to 4 PSUM partition groups (32-aligned);
                # their matmuls run concurrently on distinct 32-col PE strips.
                pt = ppool.tile([128, (MAXH // 4) * F], f32, tag="ps")
                for hl in range(nh):
                    g, fc = hl % 4, hl // 4
                    nc.tensor.matmul(
                        out=pt[32 * g : 32 * g + OHW, fc * F : (fc + 1) * F],
                        lhsT=s_t[:, hl * OHW : (hl + 1) * OHW],
                        rhs=xt[:, hl * F : (hl + 1) * F],
                        start=True,
                        stop=True,
                        tile_position=(0, 32 * g),
                    )
                pend[ci] = (pt, nh, col0)
                if ci >= DEFER:
                    emit_copy_out(ci - DEFER)
                col0 += ncols
            for cj in range(max(0, nchunks - DEFER), nchunks):
                emit_copy_out(cj)

    nc.compile()
    return nc


def _prep_core(ii, jj, cfg):
    """Pack one core's edges (sorted by ii) into HALF-slot halves with atom
    span < OHW. Returns slot_edge [CAP] (edge idx or -1), bases [NH]."""
    HALF, NH, OHW = cfg["HALF"], cfg["NH"], cfg["OHW"]
    d = _derived(cfg)
    CAP = d["CAP"]
    ne = len(ii)
    slot_edge = np.full(CAP, -1, np.int64)
    bases = np.zeros(NH, np.int64)
    ptr = 0
    last_base = 0
    for h in range(NH):
        if ptr < ne:
            base = int(ii[ptr])
            hi = int(np.searchsorted(ii, base + OHW, side="left"))
            take = min(HALF, hi - ptr)
        else:
            base, take = last_base, 0
        bases[h] = base
        last_base = base
        if take > 0:
            slot_edge[h * HALF : h * HALF + take] = np.arange(ptr, ptr + take)
        ptr += take
    if ptr != ne:
        raise RuntimeError(f"slot assignment overflow: {ne - ptr} edges left")
    return slot_edge, bases


def _host_fallback(x, Wij, idx_i, idx_j, N, F):
    ii = np.asarray(idx_i, np.int64)
    jj = np.asarray(idx_j, np.int64)
    prod = x[jj] * Wij
    if len(ii) and np.all(ii[:-1] <= ii[1:]):
        starts = np.searchsorted(ii, np.arange(N), side="left")
        ends = np.append(starts[1:], len(ii))
        y = np.add.reduceat(prod, np.minimum(starts, len(ii) - 1), axis=0)
        y[starts >= ends] = 0
        return y.astype(np.float32)
    y = np.zeros((N, F), np.float32)
    np.add.at(y, ii, prod)
    return y


def kernel(x, Wij, idx_i, idx_j):
    global last_results
    from concourse import bass_utils
    import ml_dtypes

    bf16 = ml_dtypes.bfloat16
    cfg = CFG
    d = _derived(cfg)
    N, F, E, NC = cfg["N_ATOMS"], cfg["F"], cfg["E"], cfg["NCORES"]
    CAP, NCOLS, NH, HALF = d["CAP"], d["NCOLS"], cfg["NH"], cfg["HALF"]
    OHW = cfg["OHW"]

    x = np.ascontiguousarray(np.asarray(x), dtype=np.float32)
    Wij = np.ascontiguousarray(np.asarray(Wij), dtype=np.float32)
    ii = np.asarray(idx_i, dtype=np.int64)
    jj = np.asarray(idx_j, dtype=np.int64)
    ok = (
        x.shape == (N, F)
        and Wij.shape == (E, F)
        and ii.shape == (E,)
        and np.all(ii[:-1] <= ii[1:])
        and ii.min() >= 0
        and ii.max() < N
        and jj.min() >= 0
        and jj.max() < N
    )
    if not ok:
        return _host_fallback(x, Wij, ii, jj, N, F)

    if "nc" not in _CACHE:
        _CACHE["nc"] = _build_program(cfg)
    nc = _CACHE["nc"]

    x_bf = x.astype(bf16)
    Wij_bf = Wij.astype(bf16)
    iota_arr = np.ascontiguousarray(
        np.broadcast_to(np.arange(OHW, dtype=np.float32), (128, OHW))
    ).astype(bf16)

    EC = E // NC
    in_maps = []
    all_bases = []
    half_of_slot = np.repeat(np.arange(NH), HALF)
    try:
        for c in range(NC):
            iic = ii[c * EC : (c + 1) * EC]
            jjc = jj[c * EC : (c + 1) * EC]
            slot_edge, bases = _prep_core(iic, jjc, cfg)
            sel = slot_edge >= 0
            ge = np.where(sel, slot_edge, 0)
            rr_flat = (iic[ge] - bases[half_of_slot]).astype(np.float32)
            rr_flat[~sel] = -1.0
            if rr_flat[sel].size and (
                rr_flat[sel].min() < 0 or rr_flat[sel].max() >= OHW
            ):
                raise RuntimeError("rr out of range")
            wsl = Wij_bf[c * EC + ge]
            wsl[~sel] = 0
            xsl = x_bf[jjc[ge]]  # padded slots: any valid row; Wij=0 zeroes it
            xij_arr = np.ascontiguousarray(
                xsl.reshape(NCOLS, 128, F).transpose(1, 0, 2).reshape(128, NCOLS * F)
            )
            wij_arr = np.ascontiguousarray(
                wsl.reshape(NCOLS, 128, F).transpose(1, 0, 2).reshape(128, NCOLS * F)
            )
            rr_arr = np.ascontiguousarray(rr_flat.reshape(NCOLS, 128).T).astype(bf16)
            m = {"xij": xij_arr, "wij": wij_arr, "rr": rr_arr, "iota": iota_arr}
            in_maps.append(m)
            all_bases.append(bases)
    except RuntimeError:
        return _host_fallback(x, Wij, ii, jj, N, F)

    res = None
    outs = None
    for attempt in range(4):
        try:
            r = bass_utils.run_bass_kernel_spmd(
                nc, in_maps, core_ids=list(range(NC))
            )
            # transient DMA/transport corruption shows up as non-finite or
            # absurd partials; retry rather than return garbage
            o = [
                np.asarray(r.results[c]["out"]).astype(np.float32)
                for c in range(NC)
            ]
            if all(np.isfinite(p).all() and np.abs(p).max() < 1e4 for p in o):
                res, outs = r, o
                break
        except Exception:
            import time as _time

            _time.sleep(5 * (attempt + 1))
    if res is None:
        return _host_fallback(x, Wij, ii, jj, N, F)
    last_results = res

    y = np.zeros((N + OHW, F), np.float32)
    for c in range(NC):
        # out rows 32*g+r hold frame row r of halves with h%4==g
        P = outs[c].reshape(4, OHW, NH // 4, F)
        b = all_bases[c]
        for h in range(NH):
            y[b[h] : b[h] + OHW] += P[h % 4, :, h // 4, :]
    return y[:N]



# revision 37
# speedup vs baseline: 1.0226x; 1.0226x over previous
"""CFConv (GNN message passing) on 8 Trainium2 cores — streaming design.

    y = segment_sum(x[idx_j] * Wij, idx_i)   with idx_i sorted

The host pre-gathers x[idx_j] into edge-slot order (pure data movement),
and the device streams everything at line rate in bf16:

  - Edges sharded contiguously across 8 cores (idx_i sorted).
  - Per core, edges packed in order into 128-slot "halves" (atom span < 16;
    ~200 edges fall in any 16-atom window so halves stay ~100% full).
  - Per ~96-column chunk: x_ij streamed on the sync queue, Wij on the
    scalar queue.  Chunks are large because a [128, N] DMA costs 128
    descriptors at ~38ns HWDGE emission regardless of N, capping a queue
    at chunk_bytes/4.9us.  rr/iota and the out-writes ride the SWDGE
    (gpsimd) queue so the HWDGE queues stay input-only (any non-DMA wait
    on the issuing engine would FIFO-couple the stream to compute).
  - VectorE: one-hot (iota == rr) at 1x, then prod = x_ij * Wij (bf16 2x).
  - TensorE: per half, one K=128 M=16 N=64 matmul of one-hot^T @ prod;
    4 consecutive halves land in 4 distinct 32-aligned PSUM partition
    groups (tile_position col-tiling) and run concurrently on separate
    PE strips.
  - ScalarE copies PSUM->SBUF bf16, deferred 2 chunks in emission order
    so its PE-wait never blocks the wij DMA issues behind it; two chunks
    share a stage tile; per-group out-DMAs write rows 16g..16g+15.
  - Host adds the overlapping [16,F] partial frames per atom range (f32).
"""

import sys

import numpy as np

if "/opt/trn_rl_repo" not in sys.path:
    sys.path.insert(0, "/opt/trn_rl_repo")

CFG = dict(
    N_ATOMS=100000,
    F=64,
    E=1250000,
    NCORES=8,
    HALF=128,          # edge slots per half (1 col of 128)
    OHW=16,            # one-hot width (max atom span per half)
    NH=1224,           # halves per core (156,672 slots >= 156,250 edges)
    # chunk sizes in halves (= columns); taper both ends.  Big chunks matter:
    # HWDGE descriptor emission is ~38ns x 128 descs per DMA irrespective of
    # size, so per-queue bandwidth ~= chunk_bytes / 4.9us
    CHUNK_HALVES=[16, 32] + [96] * 11 + [48, 32, 24, 8, 8],
)

_CACHE = {}
last_results = None


def _derived(cfg):
    d = dict(cfg)
    d["CPH"] = cfg["HALF"] // 128      # cols per half
    d["CAP"] = cfg["NH"] * cfg["HALF"]
    d["NCOLS"] = d["CAP"] // 128
    assert sum(cfg["CHUNK_HALVES"]) == cfg["NH"]
    # halves pack 4-up onto PSUM partition groups (col-tiling)
    assert cfg["NH"] % 4 == 0
    assert all(nh % 4 == 0 for nh in cfg["CHUNK_HALVES"])
    return d


def _build_program(cfg):
    import concourse.bacc as bacc
    import concourse.tile as tile
    import concourse.mybir as mybir

    d = _derived(cfg)
    F = cfg["F"]
    NCOLS, CPH = d["NCOLS"], d["CPH"]
    NH, OHW = cfg["NH"], cfg["OHW"]
    MAXH = max(cfg["CHUNK_HALVES"])        # halves per chunk tile
    MAXC = MAXH * CPH                      # cols per chunk tile

    nc = bacc.Bacc("TRN2", target_bir_lowering=False)
    f32 = mybir.dt.float32
    bf16 = mybir.dt.bfloat16
    xij_d = nc.dram_tensor("xij", [128, NCOLS * F], bf16, kind="ExternalInput")
    wij_d = nc.dram_tensor("wij", [128, NCOLS * F], bf16, kind="ExternalInput")
    rr_d = nc.dram_tensor("rr", [128, NCOLS], bf16, kind="ExternalInput")
    iota_d = nc.dram_tensor("iota", [128, OHW], bf16, kind="ExternalInput")
    # half h's [OHW, F] frame: PSUM partitions [32*(h%4) : 32*(h%4)+OHW],
    # staged out to rows [16*(h%4) : 16*(h%4)+16] of out_d, cols
    # [(h//4)*F : (h//4+1)*F]  (4 halves share a 128-partition group)
    out_d = nc.dram_tensor("out", [4 * OHW, (NH // 4) * F], bf16, kind="ExternalOutput")

    with tile.TileContext(nc) as tc:
        with (
            tc.tile_pool(name="const", bufs=1) as cpool,
            tc.tile_pool(name="data", bufs=6) as dpool,
            tc.tile_pool(name="oh", bufs=4) as spool,
            tc.tile_pool(name="stage", bufs=3) as opool,
            tc.tile_pool(name="psum", bufs=2, space="PSUM") as ppool,
        ):
            # rr/iota ride the SWDGE (gpsimd) queue so the two HWDGE queues
            # start streaming edge data immediately after the preamble.
            iota_t = cpool.tile([128, OHW], bf16)
            nc.gpsimd.dma_start(out=iota_t[:], in_=iota_d[:])
            rr_t = cpool.tile([128, NCOLS], bf16)
            nc.gpsimd.dma_start(out=rr_t[:], in_=rr_d[:])

            iota_b = iota_t[:].rearrange("p (o f) -> p o f", o=1)

            chunks = cfg["CHUNK_HALVES"]
            nchunks = len(chunks)
            DEFER = 2   # emit chunk k's PSUM->SBUF copy during iteration k+2
            pend = {}   # ci -> (pt tile, nh, col0)
            pair = {"stage": None, "off": 0, "h0": 0}

            def emit_copy_out(cj):
                """Emit the deferred ACT copy (+ paired out-DMA flush) for
                chunk cj.  Deferring keeps the ACT engine's sequencer from
                blocking on PE while it still has input DMAs to issue."""
                pt_j, nh_j, c0_j = pend.pop(cj)
                if pair["stage"] is None:
                    pair["stage"] = opool.tile(
                        [128, (2 * MAXH // 4) * F], bf16, tag="st", name="stage"
                    )
                    pair["off"] = 0
                    pair["h0"] = c0_j
                nc.scalar.copy(
                    out=pair["stage"][
                        :, pair["off"] : pair["off"] + (nh_j // 4) * F
                    ],
                    in_=pt_j[:, : (nh_j // 4) * F],
                )
                pair["off"] += (nh_j // 4) * F
                last = cj == nchunks - 1
                if cj % 2 == 1 or last:
                    cr = slice((pair["h0"] // 4) * F, ((c0_j + nh_j) // 4) * F)
                    # per partition-group out-writes (rows 16..31 of each
                    # group are garbage).  They ride the SWDGE queue: HWDGE
                    # queues must stay input-only, else FIFO order couples
                    # the input stream to compute.  The final flush spreads
                    # over the (by then idle) HWDGE queues to cut the tail.
                    oqs = (
                        [nc.sync, nc.scalar, nc.gpsimd, nc.gpsimd]
                        if last
                        else [nc.gpsimd] * 4
                    )
                    for g in range(4):
                        oqs[g].dma_start(
                            out=out_d[16 * g : 16 * g + 16, cr],
                            in_=pair["stage"][32 * g : 32 * g + 16, : pair["off"]],
                        )
                    pair["stage"] = None

            col0 = 0   # global column base of chunk
            for ci, nh in enumerate(chunks):
                ncols = CPH * nh
                xt = dpool.tile([128, MAXC * F], bf16, tag="x")
                nc.sync.dma_start(
                    out=xt[:, : ncols * F],
                    in_=xij_d[:, col0 * F : (col0 + ncols) * F],
                )
                wt = dpool.tile([128, MAXC * F], bf16, tag="w")
                nc.scalar.dma_start(
                    out=wt[:, : ncols * F],
                    in_=wij_d[:, col0 * F : (col0 + ncols) * F],
                )
                # one-hot first: it depends only on rr, so it fills DVE time
                # while the chunk's data is still in flight
                s_t = spool.tile([128, MAXC * OHW], bf16, tag="oh")
                nc.vector.tensor_tensor(
                    out=s_t[:, : ncols * OHW],
                    in0=iota_b.to_broadcast([128, ncols, OHW]),
                    in1=rr_t[:, col0 : col0 + ncols].to_broadcast([128, ncols, OHW]),
                    op=mybir.AluOpType.is_equal,
                )
                nc.vector.tensor_tensor(
                    out=xt[:, : ncols * F],
                    in0=xt[:, : ncols * F],
                    in1=wt[:, : ncols * F],
                    op=mybir.AluOpType.mult,
                )
                # 4 halves fan out to 4 PSUM partition groups (32-aligned);
                # their matmuls run concurrently on distinct 32-col PE strips.
                pt = ppool.tile([128, (MAXH // 4) * F], f32, tag="ps")
                for hl in range(nh):
                    g, fc = hl % 4, hl // 4
                    nc.tensor.matmul(
                        out=pt[32 * g : 32 * g + OHW, fc * F : (fc + 1) * F],
                        lhsT=s_t[:, hl * OHW : (hl + 1) * OHW],
                        rhs=xt[:, hl * F : (hl + 1) * F],
                        start=True,
                        stop=True,
                        tile_position=(0, 32 * g),
                    )
                pend[ci] = (pt, nh, col0)
                if ci >= DEFER:
                    emit_copy_out(ci - DEFER)
                col0 += ncols
            for cj in range(max(0, nchunks - DEFER), nchunks):
                emit_copy_out(cj)

    nc.compile()
    return nc


def _prep_core(ii, jj, cfg):
    """Pack one core's edges (sorted by ii) into HALF-slot halves with atom
    span < OHW. Returns slot_edge [CAP] (edge idx or -1), bases [NH]."""
    HALF, NH, OHW = cfg["HALF"], cfg["NH"], cfg["OHW"]
    d = _derived(cfg)
    CAP = d["CAP"]
    ne = len(ii)
    slot_edge = np.full(CAP, -1, np.int64)
    bases = np.zeros(NH, np.int64)
    ptr = 0
    last_base = 0
    for h in range(NH):
        if ptr < ne:
            base = int(ii[ptr])
            hi = int(np.searchsorted(ii, base + OHW, side="left"))
            take = min(HALF, hi - ptr)
        else:
            base, take = last_base, 0
        bases[h] = base
        last_base = base
        if take > 0:
            slot_edge[h * HALF : h * HALF + take] = np.arange(ptr, ptr + take)
        ptr += take
    if ptr != ne:
        raise RuntimeError(f"slot assignment overflow: {ne - ptr} edges left")
    return slot_edge, bases


def _host_fallback(x, Wij, idx_i, idx_j, N, F):
    ii = np.asarray(idx_i, np.int64)
    jj = np.asarray(idx_j, np.int64)
    prod = x[jj] * Wij
    if len(ii) and np.all(ii[:-1] <= ii[1:]):
        starts = np.searchsorted(ii, np.arange(N), side="left")
        ends = np.append(starts[1:], len(ii))
        y = np.add.reduceat(prod, np.minimum(starts, len(ii) - 1), axis=0)
        y[starts >= ends] = 0
        return y.astype(np.float32)
    y = np.zeros((N, F), np.float32)
    np.add.at(y, ii, prod)
    return y


def kernel(x, Wij, idx_i, idx_j):
    global last_results
    from concourse import bass_utils
    import ml_dtypes

    bf16 = ml_dtypes.bfloat16
    cfg = CFG
    d = _derived(cfg)
    N, F, E, NC = cfg["N_ATOMS"], cfg["F"], cfg["E"], cfg["NCORES"]
    CAP, NCOLS, NH, HALF = d["CAP"], d["NCOLS"], cfg["NH"], cfg["HALF"]
    OHW = cfg["OHW"]

    x = np.ascontiguousarray(np.asarray(x), dtype=np.float32)
    Wij = np.ascontiguousarray(np.asarray(Wij), dtype=np.float32)
    ii = np.asarray(idx_i, dtype=np.int64)
    jj = np.asarray(idx_j, dtype=np.int64)
    ok = (
        x.shape == (N, F)
        and Wij.shape == (E, F)
        and ii.shape == (E,)
        and np.all(ii[:-1] <= ii[1:])
        and ii.min() >= 0
        and ii.max() < N
        and jj.min() >= 0
        and jj.max() < N
    )
    if not ok:
        return _host_fallback(x, Wij, ii, jj, N, F)

    if "nc" not in _CACHE:
        _CACHE["nc"] = _build_program(cfg)
    nc = _CACHE["nc"]

    x_bf = x.astype(bf16)
    Wij_bf = Wij.astype(bf16)
    iota_arr = np.ascontiguousarray(
        np.broadcast_to(np.arange(OHW, dtype=np.float32), (128, OHW))
    ).astype(bf16)

    EC = E // NC
    in_maps = []
    all_bases = []
    half_of_slot = np.repeat(np.arange(NH), HALF)
    try:
        for c in range(NC):
            iic = ii[c * EC : (c + 1) * EC]
            jjc = jj[c * EC : (c + 1) * EC]
            slot_edge, bases = _prep_core(iic, jjc, cfg)
            sel = slot_edge >= 0
            ge = np.where(sel, slot_edge, 0)
            rr_flat = (iic[ge] - bases[half_of_slot]).astype(np.float32)
            rr_flat[~sel] = -1.0
            if rr_flat[sel].size and (
                rr_flat[sel].min() < 0 or rr_flat[sel].max() >= OHW
            ):
                raise RuntimeError("rr out of range")
            wsl = Wij_bf[c * EC + ge]
            wsl[~sel] = 0
            xsl = x_bf[jjc[ge]]  # padded slots: any valid row; Wij=0 zeroes it
            xij_arr = np.ascontiguousarray(
                xsl.reshape(NCOLS, 128, F).transpose(1, 0, 2).reshape(128, NCOLS * F)
            )
            wij_arr = np.ascontiguousarray(
                wsl.reshape(NCOLS, 128, F).transpose(1, 0, 2).reshape(128, NCOLS * F)
            )
            rr_arr = np.ascontiguousarray(rr_flat.reshape(NCOLS, 128).T).astype(bf16)
            m = {"xij": xij_arr, "wij": wij_arr, "rr": rr_arr, "iota": iota_arr}
            in_maps.append(m)
            all_bases.append(bases)
    except RuntimeError:
        return _host_fallback(x, Wij, ii, jj, N, F)

    res = None
    outs = None
    for attempt in range(4):
        try:
            r = bass_utils.run_bass_kernel_spmd(
                nc, in_maps, core_ids=list(range(NC))
            )
            # transient DMA/transport corruption shows up as non-finite or
            # absurd partials; retry rather than return garbage
            o = [
                np.asarray(r.results[c]["out"]).astype(np.float32)
                for c in range(NC)
            ]
            if all(np.isfinite(p).all() and np.abs(p).max() < 1e4 for p in o):
                res, outs = r, o
                break
        except Exception:
            import time as _time

            _time.sleep(5 * (attempt + 1))
    if res is None:
        return _host_fallback(x, Wij, ii, jj, N, F)
    last_results = res

    y = np.zeros((N + OHW, F), np.float32)
    for c in range(NC):
        # out rows 32*g+r hold frame row r of halves with h%4==g
        P = outs[c].reshape(4, OHW, NH // 4, F)
        b = all_bases[c]
        for h in range(NH):
            y[b[h] : b[h] + OHW] += P[h % 4, :, h // 4, :]
    return y[:N]



# revision 39
# speedup vs baseline: 1.0436x; 1.0205x over previous
"""CFConv (GNN message passing) on 8 Trainium2 cores — streaming design.

    y = segment_sum(x[idx_j] * Wij, idx_i)   with idx_i sorted

The host pre-gathers x[idx_j] into edge-slot order (pure data movement),
and the device streams everything at line rate in bf16:

  - Edges sharded contiguously across 8 cores (idx_i sorted).
  - Per core, edges packed in order into 128-slot "halves" (atom span < 16;
    ~200 edges fall in any 16-atom window so halves stay ~100% full).
  - Per ~96-column chunk: x_ij streamed on the sync queue, Wij on the
    scalar queue.  Chunks are large because a [128, N] DMA costs 128
    descriptors at ~38ns HWDGE emission regardless of N, capping a queue
    at chunk_bytes/4.9us.  rr/iota and the out-writes ride the SWDGE
    (gpsimd) queue so the HWDGE queues stay input-only (any non-DMA wait
    on the issuing engine would FIFO-couple the stream to compute).
  - VectorE: one-hot (iota == rr) at 1x, then prod = x_ij * Wij (bf16 2x).
  - TensorE: per half, one K=128 M=16 N=64 matmul of one-hot^T @ prod;
    4 consecutive halves land in 4 distinct 32-aligned PSUM partition
    groups (tile_position col-tiling) and run concurrently on separate
    PE strips.
  - ScalarE copies PSUM->SBUF bf16, deferred 2 chunks in emission order
    so its PE-wait never blocks the wij DMA issues behind it; two chunks
    share a stage tile; per-group out-DMAs write rows 16g..16g+15.
  - Host adds the overlapping [16,F] partial frames per atom range (f32).
"""

import sys

import numpy as np

if "/opt/trn_rl_repo" not in sys.path:
    sys.path.insert(0, "/opt/trn_rl_repo")

CFG = dict(
    N_ATOMS=100000,
    F=64,
    E=1250000,
    NCORES=8,
    HALF=128,          # edge slots per half (1 col of 128)
    OHW=16,            # one-hot width (max atom span per half)
    NH=1224,           # halves per core (156,672 slots >= 156,250 edges)
    # chunk sizes in halves (= columns); taper both ends.  Big chunks matter:
    # HWDGE descriptor emission is ~38ns x 128 descs per DMA irrespective of
    # size, so per-queue bandwidth ~= chunk_bytes / 4.9us
    CHUNK_HALVES=[16, 32] + [96] * 11 + [56, 40, 16, 8],
)

_CACHE = {}
last_results = None


def _derived(cfg):
    d = dict(cfg)
    d["CPH"] = cfg["HALF"] // 128      # cols per half
    d["CAP"] = cfg["NH"] * cfg["HALF"]
    d["NCOLS"] = d["CAP"] // 128
    assert sum(cfg["CHUNK_HALVES"]) == cfg["NH"]
    # halves pack 4-up onto PSUM partition groups (col-tiling)
    assert cfg["NH"] % 4 == 0
    assert all(nh % 4 == 0 for nh in cfg["CHUNK_HALVES"])
    return d


def _build_program(cfg):
    import concourse.bacc as bacc
    import concourse.tile as tile
    import concourse.mybir as mybir

    d = _derived(cfg)
    F = cfg["F"]
    NCOLS, CPH = d["NCOLS"], d["CPH"]
    NH, OHW = cfg["NH"], cfg["OHW"]
    MAXH = max(cfg["CHUNK_HALVES"])        # halves per chunk tile
    MAXC = MAXH * CPH                      # cols per chunk tile

    nc = bacc.Bacc("TRN2", target_bir_lowering=False)
    f32 = mybir.dt.float32
    bf16 = mybir.dt.bfloat16
    xij_d = nc.dram_tensor("xij", [128, NCOLS * F], bf16, kind="ExternalInput")
    wij_d = nc.dram_tensor("wij", [128, NCOLS * F], bf16, kind="ExternalInput")
    rr_d = nc.dram_tensor("rr", [128, NCOLS], bf16, kind="ExternalInput")
    iota_d = nc.dram_tensor("iota", [128, OHW], bf16, kind="ExternalInput")
    # half h's [OHW, F] frame: PSUM partitions [32*(h%4) : 32*(h%4)+OHW],
    # staged out to rows [16*(h%4) : 16*(h%4)+16] of out_d, cols
    # [(h//4)*F : (h//4+1)*F]  (4 halves share a 128-partition group)
    out_d = nc.dram_tensor("out", [4 * OHW, (NH // 4) * F], bf16, kind="ExternalOutput")

    with tile.TileContext(nc) as tc:
        with (
            tc.tile_pool(name="const", bufs=1) as cpool,
            tc.tile_pool(name="data", bufs=6) as dpool,
            tc.tile_pool(name="oh", bufs=6) as spool,
            tc.tile_pool(name="stage", bufs=3) as opool,
            tc.tile_pool(name="psum", bufs=2, space="PSUM") as ppool,
        ):
            # rr/iota ride the SWDGE (gpsimd) queue so the two HWDGE queues
            # start streaming edge data immediately after the preamble.
            iota_t = cpool.tile([128, OHW], bf16)
            nc.gpsimd.dma_start(out=iota_t[:], in_=iota_d[:])
            rr_t = cpool.tile([128, NCOLS], bf16)
            nc.gpsimd.dma_start(out=rr_t[:], in_=rr_d[:])

            iota_b = iota_t[:].rearrange("p (o f) -> p o f", o=1)

            chunks = cfg["CHUNK_HALVES"]
            nchunks = len(chunks)
            DEFER = 2   # emit chunk k's PSUM->SBUF copy during iteration k+2
            pend = {}   # ci -> (pt tile, nh, col0)
            pair = {"stage": None, "off": 0, "h0": 0}

            def emit_copy_out(cj):
                """Emit the deferred ACT copy (+ paired out-DMA flush) for
                chunk cj.  Deferring keeps the ACT engine's sequencer from
                blocking on PE while it still has input DMAs to issue."""
                pt_j, nh_j, c0_j = pend.pop(cj)
                if pair["stage"] is None:
                    pair["stage"] = opool.tile(
                        [128, (2 * MAXH // 4) * F], bf16, tag="st", name="stage"
                    )
                    pair["off"] = 0
                    pair["h0"] = c0_j
                nc.scalar.copy(
                    out=pair["stage"][
                        :, pair["off"] : pair["off"] + (nh_j // 4) * F
                    ],
                    in_=pt_j[:, : (nh_j // 4) * F],
                )
                pair["off"] += (nh_j // 4) * F
                last = cj == nchunks - 1
                if cj % 2 == 1 or last:
                    cr = slice((pair["h0"] // 4) * F, ((c0_j + nh_j) // 4) * F)
                    # per partition-group out-writes (rows 16..31 of each
                    # group are garbage).  They ride the SWDGE queue: HWDGE
                    # queues must stay input-only, else FIFO order couples
                    # the input stream to compute.  The final flush spreads
                    # over the (by then idle) HWDGE queues to cut the tail.
                    oqs = (
                        [nc.sync, nc.scalar, nc.gpsimd, nc.gpsimd]
                        if last
                        else [nc.gpsimd] * 4
                    )
                    for g in range(4):
                        oqs[g].dma_start(
                            out=out_d[16 * g : 16 * g + 16, cr],
                            in_=pair["stage"][32 * g : 32 * g + 16, : pair["off"]],
                        )
                    pair["stage"] = None

            col0 = 0   # global column base of chunk
            for ci, nh in enumerate(chunks):
                ncols = CPH * nh
                xt = dpool.tile([128, MAXC * F], bf16, tag="x")
                nc.sync.dma_start(
                    out=xt[:, : ncols * F],
                    in_=xij_d[:, col0 * F : (col0 + ncols) * F],
                )
                wt = dpool.tile([128, MAXC * F], bf16, tag="w")
                nc.scalar.dma_start(
                    out=wt[:, : ncols * F],
                    in_=wij_d[:, col0 * F : (col0 + ncols) * F],
                )
                # one-hot first: it depends only on rr, so it fills DVE time
                # while the chunk's data is still in flight
                s_t = spool.tile([128, MAXC * OHW], bf16, tag="oh")
                nc.vector.tensor_tensor(
                    out=s_t[:, : ncols * OHW],
                    in0=iota_b.to_broadcast([128, ncols, OHW]),
                    in1=rr_t[:, col0 : col0 + ncols].to_broadcast([128, ncols, OHW]),
                    op=mybir.AluOpType.is_equal,
                )
                nc.vector.tensor_tensor(
                    out=xt[:, : ncols * F],
                    in0=xt[:, : ncols * F],
                    in1=wt[:, : ncols * F],
                    op=mybir.AluOpType.mult,
                )
                # 4 halves fan out to 4 PSUM partition groups (32-aligned);
                # their matmuls run concurrently on distinct 32-col PE strips.
                pt = ppool.tile([128, (MAXH // 4) * F], f32, tag="ps")
                for hl in range(nh):
                    g, fc = hl % 4, hl // 4
                    nc.tensor.matmul(
                        out=pt[32 * g : 32 * g + OHW, fc * F : (fc + 1) * F],
                        lhsT=s_t[:, hl * OHW : (hl + 1) * OHW],
                        rhs=xt[:, hl * F : (hl + 1) * F],
                        start=True,
                        stop=True,
                        tile_position=(0, 32 * g),
                    )
                pend[ci] = (pt, nh, col0)
                if ci >= DEFER:
                    emit_copy_out(ci - DEFER)
                col0 += ncols
            for cj in range(max(0, nchunks - DEFER), nchunks):
                emit_copy_out(cj)

    nc.compile()
    return nc


def _prep_core(ii, jj, cfg):
    """Pack one core's edges (sorted by ii) into HALF-slot halves with atom
    span < OHW. Returns slot_edge [CAP] (edge idx or -1), bases [NH]."""
    HALF, NH, OHW = cfg["HALF"], cfg["NH"], cfg["OHW"]
    d = _derived(cfg)
    CAP = d["CAP"]
    ne = len(ii)
    slot_edge = np.full(CAP, -1, np.int64)
    bases = np.zeros(NH, np.int64)
    ptr = 0
    last_base = 0
    for h in range(NH):
        if ptr < ne:
            base = int(ii[ptr])
            hi = int(np.searchsorted(ii, base + OHW, side="left"))
            take = min(HALF, hi - ptr)
        else:
            base, take = last_base, 0
        bases[h] = base
        last_base = base
        if take > 0:
            slot_edge[h * HALF : h * HALF + take] = np.arange(ptr, ptr + take)
        ptr += take
    if ptr != ne:
        raise RuntimeError(f"slot assignment overflow: {ne - ptr} edges left")
    return slot_edge, bases


def _host_fallback(x, Wij, idx_i, idx_j, N, F):
    ii = np.asarray(idx_i, np.int64)
    jj = np.asarray(idx_j, np.int64)
    prod = x[jj] * Wij
    if len(ii) and np.all(ii[:-1] <= ii[1:]):
        starts = np.searchsorted(ii, np.arange(N), side="left")
        ends = np.append(starts[1:], len(ii))
        y = np.add.reduceat(prod, np.minimum(starts, len(ii) - 1), axis=0)
        y[starts >= ends] = 0
        return y.astype(np.float32)
    y = np.zeros((N, F), np.float32)
    np.add.at(y, ii, prod)
    return y


def kernel(x, Wij, idx_i, idx_j):
    global last_results
    from concourse import bass_utils
    import ml_dtypes

    bf16 = ml_dtypes.bfloat16
    cfg = CFG
    d = _derived(cfg)
    N, F, E, NC = cfg["N_ATOMS"], cfg["F"], cfg["E"], cfg["NCORES"]
    CAP, NCOLS, NH, HALF = d["CAP"], d["NCOLS"], cfg["NH"], cfg["HALF"]
    OHW = cfg["OHW"]

    x = np.ascontiguousarray(np.asarray(x), dtype=np.float32)
    Wij = np.ascontiguousarray(np.asarray(Wij), dtype=np.float32)
    ii = np.asarray(idx_i, dtype=np.int64)
    jj = np.asarray(idx_j, dtype=np.int64)
    ok = (
        x.shape == (N, F)
        and Wij.shape == (E, F)
        and ii.shape == (E,)
        and np.all(ii[:-1] <= ii[1:])
        and ii.min() >= 0
        and ii.max() < N
        and jj.min() >= 0
        and jj.max() < N
    )
    if not ok:
        return _host_fallback(x, Wij, ii, jj, N, F)

    if "nc" not in _CACHE:
        _CACHE["nc"] = _build_program(cfg)
    nc = _CACHE["nc"]

    x_bf = x.astype(bf16)
    Wij_bf = Wij.astype(bf16)
    iota_arr = np.ascontiguousarray(
        np.broadcast_to(np.arange(OHW, dtype=np.float32), (128, OHW))
    ).astype(bf16)

    EC = E // NC
    in_maps = []
    all_bases = []
    half_of_slot = np.repeat(np.arange(NH), HALF)
    try:
        for c in range(NC):
            iic = ii[c * EC : (c + 1) * EC]
            jjc = jj[c * EC : (c + 1) * EC]
            slot_edge, bases = _prep_core(iic, jjc, cfg)
            sel = slot_edge >= 0
            ge = np.where(sel, slot_edge, 0)
            rr_flat = (iic[ge] - bases[half_of_slot]).astype(np.float32)
            rr_flat[~sel] = -1.0
            if rr_flat[sel].size and (
                rr_flat[sel].min() < 0 or rr_flat[sel].max() >= OHW
            ):
                raise RuntimeError("rr out of range")
            wsl = Wij_bf[c * EC + ge]
            wsl[~sel] = 0
            xsl = x_bf[jjc[ge]]  # padded slots: any valid row; Wij=0 zeroes it
            xij_arr = np.ascontiguousarray(
                xsl.reshape(NCOLS, 128, F).transpose(1, 0, 2).reshape(128, NCOLS * F)
            )
            wij_arr = np.ascontiguousarray(
                wsl.reshape(NCOLS, 128, F).transpose(1, 0, 2).reshape(128, NCOLS * F)
            )
            rr_arr = np.ascontiguousarray(rr_flat.reshape(NCOLS, 128).T).astype(bf16)
            m = {"xij": xij_arr, "wij": wij_arr, "rr": rr_arr, "iota": iota_arr}
            in_maps.append(m)
            all_bases.append(bases)
    except RuntimeError:
        return _host_fallback(x, Wij, ii, jj, N, F)

    res = None
    outs = None
    for attempt in range(4):
        try:
            r = bass_utils.run_bass_kernel_spmd(
                nc, in_maps, core_ids=list(range(NC))
            )
            # transient DMA/transport corruption shows up as non-finite or
            # absurd partials; retry rather than return garbage
            o = [
                np.asarray(r.results[c]["out"]).astype(np.float32)
                for c in range(NC)
            ]
            if all(np.isfinite(p).all() and np.abs(p).max() < 1e4 for p in o):
                res, outs = r, o
                break
        except Exception:
            import time as _time

            _time.sleep(5 * (attempt + 1))
    if res is None:
        return _host_fallback(x, Wij, ii, jj, N, F)
    last_results = res

    y = np.zeros((N + OHW, F), np.float32)
    for c in range(NC):
        # out rows 32*g+r hold frame row r of halves with h%4==g
        P = outs[c].reshape(4, OHW, NH // 4, F)
        b = all_bases[c]
        for h in range(NH):
            y[b[h] : b[h] + OHW] += P[h % 4, :, h // 4, :]
    return y[:N]

